# revision 16
# baseline (speedup 1.0000x reference)
"""Trainium2 Bass kernel for the Koopman operator nn.Module.

Per-channel tiny MLPs (4 real channels, 6 complex-conjugate pairs, H=64,
2 hidden layers) over 65536 flattened batch elements, then a block-diagonal
Koopman update.  Pure data parallel over 8 NeuronCores (8192 elements each).

v2 strategy (weight-stationary, fp16 matmul path, transpose-free input):
  - host uploads z in three layouts: elem-major [128, 64, 16] for the final
    combine, strip-packed channel-major z1/z2 (at partition bases 0/32/64/96)
    for the on-device |z|^2, and fp16 zr rows DMA'd straight into the MLP
    input tile -- no input transposes on the tensor engine
  - all matmuls fp16 (1 cycle/row) with fp32 PSUM accumulation
  - 4 quarters x (L0 / hid0 / hid1 / final phases across all 5 pair-blocks):
    consecutive matmuls share stationary weights and the program order lets
    pair j's ReLUs drain while pairs j+1.. stream, keeping the PE p-state
    ramped at 2.4 GHz
  - ReLUs round-robin over DVE / Act / GpSimd (three engines)
  - final-layer outputs go back to elem-major via 4 PE transposes per chunk;
    exp/sin/combine run as a handful of big batched ops at the end
    (sin(x+pi/2) for cos); activation table loads ~2 for the whole kernel
"""

import numpy as np

NR, NCC, L, H = 4, 6, 2, 64
B, S, C = 32, 2048, 16
NCORES = 8
F_CORE = B * S // NCORES        # 8192 elements per core
CHUNK = 512                     # elements per matmul chunk (one PSUM bank)
NCH = F_CORE // CHUNK           # 16 chunks
NQ = 4                          # quarters
KPQ = NCH // NQ                 # 4 chunks per quarter
NSTRIP = 2                      # xcat partition bands at bases 0 and 64
STRIP = F_CORE // NSTRIP        # 4096 elements per strip
CPS = STRIP // CHUNK            # 8 chunks per strip

HALF_PI = float(np.pi / 2)

# wcat column layout (fp16): w0(5x128) | wm0(5x128) | wm1(5x128) | wl(5x128)
# | ident(128)
W0_OFF, WM0_OFF, WM1_OFF, WL_OFF, ID_OFF = 0, 640, 1280, 1920, 2560
WCAT_COLS = 2688
# bcat column layout (fp32): b0(5) | bm0(5) | bm1(5) | bias128(1) | pi/2(1)
BCAT_COLS = 17

_cached_nc = {}


def _build(zero_bias):
    import concourse.tile as tile
    from concourse import bacc, mybir

    f32 = mybir.dt.float32
    f16 = mybir.dt.float16
    RELU = mybir.ActivationFunctionType.Relu
    IDENT = mybir.ActivationFunctionType.Identity
    SQUARE = mybir.ActivationFunctionType.Square
    EXP = mybir.ActivationFunctionType.Exp
    SIN = mybir.ActivationFunctionType.Sin
    ADD = mybir.AluOpType.add
    MAX = mybir.AluOpType.max

    nc = bacc.Bacc("TRN2", target_bir_lowering=False, debug=False,
                   num_devices=NCORES)

    zin_d = nc.dram_tensor("zin", [128, NCH, KPQ, C], f32,
                           kind="ExternalInput").ap()
    wcat_d = nc.dram_tensor("wcat", [128, WCAT_COLS], f16,
                            kind="ExternalInput").ap()
    bcat_d = nc.dram_tensor("bcat", [128, BCAT_COLS], f32,
                            kind="ExternalInput").ap()
    out_d = nc.dram_tensor("out", [128, NCH, KPQ, C], f32,
                           kind="ExternalOutput").ap()

    with tile.TileContext(nc) as tc:
        with (
            tc.tile_pool(name="singles", bufs=1) as singles,
            tc.tile_pool(name="scratch", bufs=1) as scratch,
            tc.tile_pool(name="hps", bufs=1) as hps,
            tc.tile_pool(name="pshid", bufs=5, space="PSUM") as pshid,
            tc.tile_pool(name="pstk", bufs=1, space="PSUM") as pstk,
            tc.tile_pool(name="pstp", bufs=1, space="PSUM") as pstp,
        ):
            # ---- uploads ----
            wcat = singles.tile([128, WCAT_COLS], f16, tag="wcat")
            nc.sync.dma_start(out=wcat[:, ID_OFF:ID_OFF + 128],
                              in_=wcat_d[:, ID_OFF:ID_OFF + 128])
            bcat = singles.tile([128, BCAT_COLS], f32, tag="bcat")
            nc.sync.dma_start(out=bcat, in_=bcat_d)
            nc.sync.dma_start(out=wcat[:, 0:ID_OFF], in_=wcat_d[:, 0:ID_OFF])
            zin = singles.tile([128, NCH, KPQ, C], f32, tag="zin")

            # ---- x = [zmag(6) ; zr(4)]: build elem-major per quarter,
            # PE-transpose pairs of chunks to partition bases 0/32, one
            # engine copy into xcat per pair
            xcat = singles.tile([42, NCH * CHUNK // 2], f16, tag="xcat")
            x_nat = singles.tile([128, NCH, KPQ, 10], f16, tag="x_nat")

            def emit_xnat(qx):
                qs = slice(qx * KPQ, (qx + 1) * KPQ)
                nc.sync.dma_start(out=zin[:, qs], in_=zin_d[:, qs])
                sq1 = scratch.tile([128, KPQ, KPQ, 6], f32, tag="sq1")
                sq2 = scratch.tile([128, KPQ, KPQ, 6], f32, tag="sq2")
                z1_vv = zin[:, qs, :, 4:16:2]
                z2_vv = zin[:, qs, :, 5:16:2]
                nc.vector.tensor_mul(sq1, z1_vv, z1_vv)
                nc.gpsimd.tensor_mul(sq2, z2_vv, z2_vv)
                nc.vector.tensor_add(x_nat[:, qs, :, 0:6], sq1, sq2)
                nc.vector.tensor_copy(x_nat[:, qs, :, 6:10],
                                      zin[:, qs, :, 0:4])

            ident = wcat[:, ID_OFF:ID_OFF + 128]
            bias128 = bcat[:, 15:16]

            # full-width staging for the post phase
            t_all = singles.tile([128, NCH, CHUNK], f16, tag="t_all")
            o_full = singles.tile([128, NCH, KPQ, C], f32, tag="o_full")

            # ReLU engine round-robin: weighted DVE/Act/Pool
            relu_seq = []

            def relu(h, ps, bias_ap):
                i = len(relu_seq) % 2
                relu_seq.append(0)
                if i == 0:
                    if zero_bias:
                        nc.vector.tensor_single_scalar(h, ps, 0.0, MAX)
                    else:
                        nc.vector.tensor_scalar(h, ps, bias_ap, 0.0, ADD, MAX)
                else:
                    if zero_bias:
                        nc.scalar.activation(h, ps, RELU)
                    else:
                        nc.scalar.activation(h, ps, RELU, bias=bias_ap)

            def emit_psx(g2):
                # transpose a pair of chunks of x_nat into channel-major rows
                # stacked at partition bases 0/32 of one PSUM bank, then one
                # engine copy moves both into xcat
                psx = pstp.tile([128, CHUNK], f16, tag="tpx")
                for i, k in enumerate((2 * g2, 2 * g2 + 1)):
                    for g in range(KPQ):
                        nc.tensor.transpose(
                            psx[32 * i:32 * i + 10, g * 128:(g + 1) * 128],
                            x_nat[:, k, g], ident)
                nc.vector.tensor_copy(
                    xcat[:, g2 * CHUNK:(g2 + 1) * CHUNK], psx[0:42])

            # ---- MLP: 4 quarters, weight-stationary phases ----
            emit_xnat(0)
            for q in range(NQ):
                h0, h1, h2 = {}, {}, {}
                # layer 0 (chunk-outer so quarter 0 streams as soon as the
                # first chunks of x are transposed; prefetch next quarter's x)
                for kk in range(KPQ):
                    k = q * KPQ + kk
                    if q == 0 and kk % 2 == 0:
                        emit_psx(k // 2)
                    m, cg = k % 2, k // 2
                    rsk = slice(32 * m, 32 * m + 10)
                    cc = cg * CHUNK
                    for j in range(5):
                        w = wcat[rsk, W0_OFF + j * 128:W0_OFF + (j + 1) * 128]
                        ps = pshid.tile([128, CHUNK], f32, tag="ps")
                        nc.tensor.matmul(
                            ps, w, xcat[rsk, cc:cc + CHUNK],
                            start=True, stop=True)
                        h = hps.tile([128, CHUNK], f16, tag=f"h0_{j}_{kk}")
                        relu(h, ps, bcat[:, j:j + 1])
                        h0[j, kk] = h
                if q < NQ - 1:
                    emit_xnat(q + 1)
                    emit_psx(2 * (q + 1))
                    emit_psx(2 * (q + 1) + 1)
                # hidden layers
                for l, (off, hin, hout) in enumerate(
                        ((WM0_OFF, h0, h1), (WM1_OFF, h1, h2))):
                    for j in range(5):
                        w = wcat[:, off + j * 128:off + (j + 1) * 128]
                        b = bcat[:, 5 + 5 * l + j:6 + 5 * l + j]
                        for kk in range(KPQ):
                            ps = pshid.tile([128, CHUNK], f32, tag="ps")
                            nc.tensor.matmul(ps, w, hin[j, kk],
                                             start=True, stop=True)
                            h = hps.tile([128, CHUNK], f16,
                                         tag=f"h{l + 1}_{j}_{kk}")
                            relu(h, ps, b)
                            hout[j, kk] = h
                # final layer: disjoint output rows per j; per chunk-pair so
                # only 2 stk banks stay live
                for pp in range(KPQ // 2):
                    stks = []
                    for j in range(5):
                        w = wcat[:, WL_OFF + j * 128:WL_OFF + (j + 1) * 128]
                        for u in range(2):
                            if j == 0:
                                stk_t = pstk.tile([128, CHUNK], f32,
                                                  tag=f"stk_{u}")
                                stks.append(stk_t)
                            nc.tensor.matmul(stks[u], w, h2[j, 2 * pp + u],
                                             start=(j == 0), stop=(j == 4))
                    # post: +bias, fp16, transpose to elem-major
                    for u in range(2):
                        k = q * KPQ + 2 * pp + u
                        sstk = hps.tile([128, CHUNK], f16, tag=f"sstk_{u}")
                        nc.scalar.activation(sstk, stks[u], IDENT,
                                             bias=bias128)
                        tp = pstp.tile([128, CHUNK], f16, tag="tpx")
                        for g in range(KPQ):
                            nc.tensor.transpose(
                                tp[:, g * 128:(g + 1) * 128],
                                sstk[:, g * 128:(g + 1) * 128], ident)
                        nc.vector.tensor_copy(t_all[:, k], tp)

                # post batches: chunks [0,8) after q=1, [8,12) after q=2,
                # [12,14) and [14,16) inside q=3; overlaps later MLP work
                batches = []
                if q == 1:
                    batches = [slice(0, 8)]
                elif q == 2:
                    batches = [slice(8, 12)]
                elif q == 3:
                    batches = [slice(12, 14), slice(14, 16)]
                for hs in batches:
                    nb = hs.stop - hs.start
                    t4 = t_all.rearrange("p k (g c) -> p k g c",
                                         g=KPQ, c=128)
                    lamT = t4[:, hs, :, 0:4]
                    muT = t4[:, hs, :, 32:38]
                    omT = t4[:, hs, :, 64:70]
                    e_f = scratch.tile([128, nb, KPQ, 6], f32, tag="e_f")
                    cs_f = scratch.tile([128, nb, KPQ, 6], f32, tag="cs_f")
                    sn_f = scratch.tile([128, nb, KPQ, 6], f32, tag="sn_f")
                    nc.scalar.activation(e_f, muT, EXP)
                    nc.scalar.activation(cs_f, omT, SIN, bias=bcat[:, 16:17])
                    nc.scalar.activation(sn_f, omT, SIN)
                    mc_f = scratch.tile([128, nb, KPQ, 6], f32, tag="mc_f")
                    ms_f = scratch.tile([128, nb, KPQ, 6], f32, tag="ms_f")
                    nc.gpsimd.tensor_mul(mc_f, e_f, cs_f)
                    nc.vector.tensor_mul(ms_f, e_f, sn_f)
                    zr_v = zin[:, hs, :, 0:4]
                    z1_v = zin[:, hs, :, 4:16:2]
                    z2_v = zin[:, hs, :, 5:16:2]
                    ov = o_full[:, hs]
                    t1f = scratch.tile([128, nb, KPQ, 6], f32, tag="t1f")
                    t2f = scratch.tile([128, nb, KPQ, 6], f32, tag="t2f")
                    nc.vector.tensor_mul(ov[:, :, :, 0:4], zr_v, lamT)
                    nc.gpsimd.tensor_mul(t1f, z1_v, mc_f)
                    nc.vector.tensor_mul(t2f, z2_v, ms_f)
                    nc.vector.tensor_add(ov[:, :, :, 4:16:2], t1f, t2f)
                    nc.vector.tensor_mul(t1f, z2_v, mc_f)
                    nc.vector.tensor_mul(t2f, z1_v, ms_f)
                    nc.vector.tensor_sub(ov[:, :, :, 5:16:2], t1f, t2f)
                    nc.sync.dma_start(out=out_d[:, hs], in_=ov)



    nc.compile()
    return nc


def _pack_weights(i):
    """Pack per-channel weights into the fused fp16 wcat / fp32 bcat blocks."""
    f32, f16 = np.float32, np.float16
    W0_r, b0_r = np.asarray(i["W0_r"], f32), np.asarray(i["b0_r"], f32)
    Wm_r, bm_r = np.asarray(i["Wm_r"], f32), np.asarray(i["bm_r"], f32)
    Wl_r, bl_r = np.asarray(i["Wl_r"], f32), np.asarray(i["bl_r"], f32)
    W0_c, b0_c = np.asarray(i["W0_c"], f32), np.asarray(i["b0_c"], f32)
    Wm_c, bm_c = np.asarray(i["Wm_c"], f32), np.asarray(i["bm_c"], f32)
    Wl_c, bl_c = np.asarray(i["Wl_c"], f32), np.asarray(i["bl_c"], f32)

    wcat = np.zeros((128, WCAT_COLS), f16)
    bcat = np.zeros((128, BCAT_COLS), f32)
    for j in range(5):
        if j < 2:
            a, b = 2 * j, 2 * j + 1
            W0, b0, Wm, bm = W0_r, b0_r, Wm_r, bm_r
            xra, xrb = 6 + a, 6 + b          # zr rows of x
        else:
            a, b = 2 * (j - 2), 2 * (j - 2) + 1
            W0, b0, Wm, bm = W0_c, b0_c, Wm_c, bm_c
            xra, xrb = a, b                  # zmag rows of x
        # layer 0, replicated at partition bases 0/32
        for m in range(2):
            wcat[32 * m + xra, W0_OFF + j * 128:W0_OFF + j * 128 + 64] = W0[a]
            wcat[32 * m + xrb, W0_OFF + j * 128 + 64:W0_OFF + (j + 1) * 128] \
                = W0[b]
        bcat[0:64, j] = b0[a]
        bcat[64:128, j] = b0[b]
        # hidden layers, block diagonal
        for l, off in enumerate((WM0_OFF, WM1_OFF)):
            wcat[0:64, off + j * 128:off + j * 128 + 64] = Wm[l, a]
            wcat[64:128, off + j * 128 + 64:off + (j + 1) * 128] = Wm[l, b]
            bcat[0:64, 5 + 5 * l + j] = bm[l, a]
            bcat[64:128, 5 + 5 * l + j] = bm[l, b]
        # final layer -> rows 0-3 lam, 32-37 mu, 64-69 om
        wo = WL_OFF + j * 128
        if j < 2:
            wcat[0:64, wo + 2 * j] = Wl_r[a][:, 0]
            wcat[64:128, wo + 2 * j + 1] = Wl_r[b][:, 0]
        else:
            jc = j - 2
            wcat[0:64, wo + 32 + 2 * jc] = Wl_c[a][:, 0]
            wcat[64:128, wo + 33 + 2 * jc] = Wl_c[b][:, 0]
            wcat[0:64, wo + 64 + 2 * jc] = Wl_c[a][:, 1]
            wcat[64:128, wo + 65 + 2 * jc] = Wl_c[b][:, 1]
    wcat[:, ID_OFF:ID_OFF + 128] = np.eye(128, dtype=f16)
    bcat[:, 16] = HALF_PI
    bcat[0:4, 15] = bl_r[:, 0]
    bcat[32:38, 15] = bl_c[:, 0]
    bcat[64:70, 15] = bl_c[:, 1]
    return {"wcat": wcat, "bcat": bcat}


def _pack_z(z_core):
    """Per-core z [8192, 16] -> elem-major zin DRAM layout."""
    zc = np.asarray(z_core, np.float32)
    zin = np.ascontiguousarray(
        zc.reshape(64, 128, C).transpose(1, 0, 2)).reshape(128, NCH, KPQ, C)
    return {"zin": zin}


def kernel(**inputs):
    zero_bias = all(
        not np.any(np.asarray(inputs[k]))
        for k in ("b0_r", "bm_r", "bl_r", "b0_c", "bm_c", "bl_c"))
    if zero_bias not in _cached_nc:
        _cached_nc[zero_bias] = _build(zero_bias)
    nc = _cached_nc[zero_bias]

    from concourse.bass_utils import run_bass_kernel_spmd

    weights = _pack_weights(inputs)
    z = np.asarray(inputs["z"], np.float32).reshape(NCORES, F_CORE, C)
    in_maps = [dict(weights, **_pack_z(z[i])) for i in range(NCORES)]
    res = run_bass_kernel_spmd(nc, in_maps, core_ids=list(range(NCORES)))
    outs = [
        np.asarray(res.results[i]["out"])
        .reshape(128, 64, C).transpose(1, 0, 2).reshape(F_CORE, C)
        for i in range(NCORES)
    ]
    return np.concatenate(outs, axis=0).reshape(B, S, C)
